# revision 1
# baseline (speedup 1.0000x reference)
"""Trainium2 Bass kernel for the 6-layer post-LN transformer encoder.

Data-parallel over batch: 8 NeuronCores x 2 batches each, weights replicated,
no collectives.  Activations are kept feature-major ``hT[d, token]`` so every
linear layer runs with weight tiles stationary; LayerNorm statistics are
computed with ones-vector matmuls on the PE.  Matmuls run in float32r
(full-rate fp32, ~1e-4 rounding) except Q/K score matmuls which run in bf16.

The reference faithfully replicates torch's buggy ``.view(B*H, -1, Dh)`` head
split, so attention operates on 128 "pseudo-groups" (batch x 64-token block)
of 512 pseudo-positions j = (s%64)*8 + h, and group ``i`` uses the padding
mask of batch ``i % B``.  Scores are computed transposed (pseudo-keys on
partitions, block order jk' = hk*64 + smk) so the padding mask folds into the
Exp bias and the softmax denominator comes from a ones-column on V.
"""

import os
import sys

import numpy as np

for _p in ("/opt/trn_rl_repo", "/root/.axon_site/_ro/trn_rl_repo"):
    if os.path.isdir(_p) and _p not in sys.path:
        sys.path.append(_p)

import concourse.bass as bass
import concourse.mybir as mybir
from concourse import bacc
from concourse.tile import TileContext
from concourse.bass_utils import run_bass_kernel_spmd
from concourse.masks import make_identity

B, S, D, H, Dh, F, L, V = 16, 512, 512, 8, 64, 2048, 6, 32000
NCORES = 8
BPC = B // NCORES          # batches per core
NT = BPC * S               # tokens per core
DT = D // 128              # d-dim partition tiles
FT = F // 128              # ffn-dim partition tiles
NG = BPC * 8               # pseudo attention groups per core
f32 = mybir.dt.float32
f32r = mybir.dt.float32r
bf16 = mybir.dt.bfloat16
AF = mybir.ActivationFunctionType

_CACHE = {}
_UID = [0]


def _nm(p):
    _UID[0] += 1
    return f"{p}{_UID[0]}"


def _build():
    nc = bacc.Bacc(None, target_bir_lowering=False)

    IDX = nc.dram_tensor("IDX", [128, NT // 16], mybir.dt.int16, kind="ExternalInput")
    PEM = nc.dram_tensor("PEM", [DT, 128, NT], f32, kind="ExternalInput")
    KB = nc.dram_tensor("KB", [128, 16 * 4], f32, kind="ExternalInput")
    EMB = nc.dram_tensor("EMB", [V, D], f32, kind="ExternalInput")
    WQ = nc.dram_tensor("WQ", [L, DT, DT, 128, 128], f32r, kind="ExternalInput")
    WK = nc.dram_tensor("WK", [L, DT, DT, 128, 128], f32r, kind="ExternalInput")
    WV = nc.dram_tensor("WV", [L, DT, 128, D], f32r, kind="ExternalInput")
    WO = nc.dram_tensor("WO", [L, DT, DT, 128, 128], f32r, kind="ExternalInput")
    W1 = nc.dram_tensor("W1", [L, DT, FT, 128, 128], f32r, kind="ExternalInput")
    W2 = nc.dram_tensor("W2", [L, FT, DT, 128, 128], f32r, kind="ExternalInput")
    # biases / gains, host-tiled to [L, 128, ntiles]
    BO = nc.dram_tensor("BO", [L, 128, DT], f32, kind="ExternalInput")
    B1 = nc.dram_tensor("B1", [L, 128, FT], f32, kind="ExternalInput")
    B2 = nc.dram_tensor("B2", [L, 128, DT], f32, kind="ExternalInput")
    G1 = nc.dram_tensor("G1", [L, 128, DT], f32, kind="ExternalInput")
    E1 = nc.dram_tensor("E1", [L, 128, DT], f32, kind="ExternalInput")
    G2 = nc.dram_tensor("G2", [L, 128, DT], f32, kind="ExternalInput")
    E2 = nc.dram_tensor("E2", [L, 128, DT], f32, kind="ExternalInput")
    OUT = nc.dram_tensor("OUT", [NT, D], f32, kind="ExternalOutput")

    from contextlib import ExitStack

    with TileContext(nc) as tc:
        with ExitStack() as st:
            act_pool = st.enter_context(tc.tile_pool(name="act", bufs=8))
            emb_pool = st.enter_context(tc.tile_pool(name="emb", bufs=1))
            tmp_pool = st.enter_context(tc.tile_pool(name="tmp", bufs=3))
            ctx_pool = st.enter_context(tc.tile_pool(name="ctx", bufs=4))
            qk_pool = st.enter_context(tc.tile_pool(name="qk", bufs=1))
            v_pool = st.enter_context(tc.tile_pool(name="vst", bufs=16))
            e_pool = st.enter_context(tc.tile_pool(name="ebuf", bufs=3))
            eo_pool = st.enter_context(tc.tile_pool(name="eodd", bufs=3))
            f_pool = st.enter_context(tc.tile_pool(name="fbuf", bufs=4))
            uw_pool = st.enter_context(tc.tile_pool(name="uw", bufs=1))
            wt_pool = st.enter_context(tc.tile_pool(name="wt", bufs=12))
            wv_pool = st.enter_context(tc.tile_pool(name="wv4", bufs=4))
            sm_pool = st.enter_context(tc.tile_pool(name="small", bufs=2))
            cst_pool = st.enter_context(tc.tile_pool(name="cst", bufs=1))
            pbig = st.enter_context(tc.tile_pool(name="pbig", bufs=2, space="PSUM"))
            pacc = st.enter_context(tc.tile_pool(name="pacc", bufs=4, space="PSUM"))
            # ---- constants ----
            ident = cst_pool.tile([128, 128], f32, tag="identf")
            make_identity(nc, ident[:, :])
            identr = cst_pool.tile([128, 128], f32r, tag="identr")
            nc.vector.tensor_copy(identr[:, :], ident[:, :])
            ones_f = cst_pool.tile([128, 9], f32, tag="onesf")
            nc.gpsimd.memset(ones_f[:, :], 1.0)
            ones_r = cst_pool.tile([128, 1], f32r, tag="ones")
            nc.vector.tensor_copy(ones_r[:, :], ones_f[:, 0:1])
            kb_sb = cst_pool.tile([128, 64], f32, tag="kb")
            nc.sync.dma_start(kb_sb[:, :], KB[:, :])
            idx_sb = cst_pool.tile([128, NT // 16], mybir.dt.int16, tag="idx")
            nc.sync.dma_start(idx_sb[:, :], IDX[:, :])
            eps_sb = cst_pool.tile([1, 1], f32, tag="eps")
            nc.gpsimd.memset(eps_sb[:, :], 1e-5)

            # ---- embedding: gather + transpose + pe add ----
            hT = [act_pool.tile([128, NT], f32r, tag="act", name=_nm("hT")) for _ in range(DT)]
            pem_sb = []
            for dt in range(DT):
                p = act_pool.tile([128, NT], f32, tag="act")
                nc.sync.dma_start(p[:, :], PEM[dt, :, :])
                pem_sb.append(p)
            for half in range(2):
                g_sb = emb_pool.tile([128, NT // 256, D], f32, tag="gsb")
                nc.gpsimd.dma_gather(
                    g_sb[:, :, :], EMB[:, :],
                    idx_sb[:, half * (NT // 32):(half + 1) * (NT // 32)],
                    NT // 2, NT // 2, D,
                )
                for ch in range(NT // 256):
                    c = half * (NT // 256) + ch
                    for dt in range(DT):
                        pt = pacc.tile([128, 128], f32, tag="pacc")
                        nc.tensor.transpose(
                            pt[:, :], g_sb[:, ch, dt * 128:(dt + 1) * 128], ident[:, :]
                        )
                        nc.vector.tensor_add(
                            hT[dt][:, c * 128:(c + 1) * 128],
                            pt[:, :],
                            pem_sb[dt][:, c * 128:(c + 1) * 128],
                        )

            # ---- layers ----
            for l in range(L):
                # --- load per-layer bias tiles ---
                bo_sb = sm_pool.tile([128, DT], f32, tag="bo")
                nc.sync.dma_start(bo_sb[:, :], BO[l, :, :])
                b1_sb = sm_pool.tile([128, FT], f32, tag="b1")
                nc.sync.dma_start(b1_sb[:, :], B1[l, :, :])
                b2_sb = sm_pool.tile([128, DT], f32, tag="b2")
                nc.sync.dma_start(b2_sb[:, :], B2[l, :, :])
                g1_sb = sm_pool.tile([128, DT], f32, tag="g1")
                nc.sync.dma_start(g1_sb[:, :], G1[l, :, :])
                e1_sb = sm_pool.tile([128, DT], f32, tag="e1")
                nc.sync.dma_start(e1_sb[:, :], E1[l, :, :])
                g2_sb = sm_pool.tile([128, DT], f32, tag="g2")
                nc.sync.dma_start(g2_sb[:, :], G2[l, :, :])
                e2_sb = sm_pool.tile([128, DT], f32, tag="e2")
                nc.sync.dma_start(e2_sb[:, :], E2[l, :, :])

                # --- Q / K projections into head-stacked bf16 [64, H*NT] ---
                q_stack = qk_pool.tile([64, H * NT], bf16, tag="qs")
                k_stack = qk_pool.tile([64, H * NT], bf16, tag="ks")
                for W_dram, stack in ((WQ, q_stack), (WK, k_stack)):
                    for mt in range(DT):
                        ps = pbig.tile([128, NT], f32, tag="pbig")
                        for kt in range(DT):
                            w_t = wt_pool.tile([128, 128], f32r, tag="wt")
                            nc.sync.dma_start(w_t[:, :], W_dram[l, kt, mt, :, :])
                            for nch in range(NT // 512):
                                nc.tensor.matmul(
                                    ps[:, nch * 512:(nch + 1) * 512],
                                    w_t[:, :],
                                    hT[kt][:, nch * 512:(nch + 1) * 512],
                                    start=(kt == 0),
                                    stop=(kt == DT - 1),
                                )
                        nc.scalar.copy(
                            stack[:, (2 * mt) * NT:(2 * mt + 1) * NT], ps[0:64, :]
                        )
                        nc.scalar.copy(
                            stack[:, (2 * mt + 1) * NT:(2 * mt + 2) * NT],
                            ps[64:128, :],
                        )

                # --- V projection into token-major with ones column ---
                wv_sb = []
                for kt in range(DT):
                    w_t = wv_pool.tile([128, D], f32r, tag="wv")
                    nc.sync.dma_start(w_t[:, :], WV[l, kt, :, :])
                    wv_sb.append(w_t)
                v_st = []                      # one [64, H*65] tile per group
                for tt in range(NT // 128):
                    ps = pacc.tile([128, 512], f32, tag="pacc")
                    for kt in range(DT):
                        nc.tensor.matmul(
                            ps[:, :],
                            hT[kt][:, tt * 128:(tt + 1) * 128],
                            wv_sb[kt][:, :],
                            start=(kt == 0),
                            stop=(kt == DT - 1),
                        )
                    for half in range(2):
                        vt = v_pool.tile([64, H * 65], f32r, tag="vst")
                        nc.scalar.copy(
                            vt.rearrange("p (h e) -> p h e", h=H)[:, :, 0:64],
                            ps[half * 64:half * 64 + 64, :],
                        )
                        nc.vector.tensor_copy(
                            vt.rearrange("p (h e) -> p h e", h=H)[:, :, 64:65],
                            ones_f[0:64, 0:8].rearrange("p (h o) -> p h o", o=1),
                        )
                        v_st.append(vt)

                # --- attention per pseudo-group ---
                ctxT = [ctx_pool.tile([128, NT], f32r, tag="ctx", name=_nm("ctxT")) for _ in range(DT)]
                q_v = q_stack.rearrange("p (h t) -> p h t", h=H)
                for g in range(NG):
                    tb = g * 64                       # local token base
                    m = g % 16                        # mask row: 8*(b%2)+gb == g%16
                    sp = [pacc.tile([128, 512], f32, tag="pacc", name=_nm("sp")) for _ in range(4)]
                    rhs = q_v[:, :, tb:tb + 64]
                    for hk in range(H):
                        kt2 = hk // 2
                        nc.tensor.matmul(
                            sp[kt2][(hk % 2) * 64:(hk % 2) * 64 + 64, :],
                            k_stack[:, hk * NT + tb:hk * NT + tb + 64],
                            rhs,
                            start=True,
                            stop=True,
                        )
                    e4 = []
                    eo = []
                    for kt2 in range(4):
                        e_t = e_pool.tile([128, 512], f32r, tag="e4")
                        nc.scalar.activation(
                            e_t[:, :],
                            sp[kt2][:, :],
                            AF.Exp,
                            bias=kb_sb[:, m * 4 + kt2:m * 4 + kt2 + 1],
                            scale=1.0,
                        )
                        e4.append(e_t)
                        et = eo_pool.tile([64, 512], f32r, tag="eo")
                        nc.vector.tensor_copy(et[:, :], e_t[64:128, :])
                        eo.append(et)
                    cps = pbig.tile([128, 512], f32, tag="pbig", name=_nm("cps"))
                    for hk in range(H):
                        rhs_e = e4[hk // 2][0:64, :] if hk % 2 == 0 else eo[hk // 2][:, :]
                        nc.tensor.matmul(
                            cps[0:65, :],
                            v_st[g][:, hk * 65:hk * 65 + 65],
                            rhs_e,
                            start=(hk == 0),
                            stop=(hk == H - 1),
                        )
                    rec = sm_pool.tile([1, 512], f32, tag="rec", bufs=2)
                    nc.vector.reciprocal(rec[:, :], cps[64:65, :])
                    rb = uw_pool.tile([64, 512], f32, tag="rb")
                    nc.gpsimd.partition_broadcast(rb[:, :], rec[0:1, :])
                    for hq in range(H):
                        nc.vector.tensor_mul(
                            ctxT[hq // 2][(hq % 2) * 64:(hq % 2) * 64 + 64, tb:tb + 64],
                            cps[0:64, hq * 64:(hq + 1) * 64],
                            rb[:, hq * 64:(hq + 1) * 64],
                        )

                # --- Wo + bias + residual -> h_pre ---
                h_pre = [act_pool.tile([128, NT], f32r, tag="act", name=_nm("hpre")) for _ in range(DT)]
                for mt in range(DT):
                    ps = pbig.tile([128, NT], f32, tag="pbig")
                    for kt in range(DT):
                        w_t = wt_pool.tile([128, 128], f32r, tag="wt")
                        nc.sync.dma_start(w_t[:, :], WO[l, kt, mt, :, :])
                        for nch in range(NT // 512):
                            nc.tensor.matmul(
                                ps[:, nch * 512:(nch + 1) * 512],
                                w_t[:, :],
                                ctxT[kt][:, nch * 512:(nch + 1) * 512],
                                start=(kt == 0),
                                stop=(kt == DT - 1),
                            )
                    t_o = tmp_pool.tile([128, NT], f32r, tag="tmp")
                    nc.scalar.activation(
                        t_o[:, :], ps[:, :], AF.Identity,
                        bias=bo_sb[:, mt:mt + 1], scale=1.0,
                    )
                    nc.vector.tensor_add(h_pre[mt][:, :], t_o[:, :], hT[mt][:, :])

                h1 = _layernorm(nc, act_pool, tmp_pool, uw_pool, sm_pool, pacc,
                                ones_r, eps_sb, h_pre, g1_sb, e1_sb)

                # --- FFN ---
                h_pre2 = [act_pool.tile([128, NT], f32r, tag="act", name=_nm("hpre2")) for _ in range(DT)]
                for nch in range(NT // 512):
                    o2ps = [pacc.tile([128, 512], f32, tag="pacc", name=_nm("o2ps")) for _ in range(DT)]
                    for mtf in range(FT):
                        ps = pbig.tile([128, 512], f32, tag="pbig")
                        for kt in range(DT):
                            w_t = wt_pool.tile([128, 128], f32r, tag="wt")
                            nc.sync.dma_start(w_t[:, :], W1[l, kt, mtf, :, :])
                            nc.tensor.matmul(
                                ps[:, :],
                                w_t[:, :],
                                h1[kt][:, nch * 512:(nch + 1) * 512],
                                start=(kt == 0),
                                stop=(kt == DT - 1),
                            )
                        f_t = f_pool.tile([128, 512], f32r, tag="f")
                        nc.scalar.activation(
                            f_t[:, :], ps[:, :], AF.Relu,
                            bias=b1_sb[:, mtf:mtf + 1], scale=1.0,
                        )
                        for mtd in range(DT):
                            w_t2 = wt_pool.tile([128, 128], f32r, tag="wt")
                            nc.sync.dma_start(w_t2[:, :], W2[l, mtf, mtd, :, :])
                            nc.tensor.matmul(
                                o2ps[mtd][:, :],
                                w_t2[:, :],
                                f_t[:, :],
                                start=(mtf == 0),
                                stop=(mtf == FT - 1),
                            )
                    for mtd in range(DT):
                        t_o = tmp_pool.tile([128, 512], f32r, tag="tmp")
                        nc.scalar.activation(
                            t_o[:, :], o2ps[mtd][:, :], AF.Identity,
                            bias=b2_sb[:, mtd:mtd + 1], scale=1.0,
                        )
                        nc.vector.tensor_add(
                            h_pre2[mtd][:, nch * 512:(nch + 1) * 512],
                            t_o[:, :],
                            h1[mtd][:, nch * 512:(nch + 1) * 512],
                        )

                hT = _layernorm(nc, act_pool, tmp_pool, uw_pool, sm_pool, pacc,
                                ones_r, eps_sb, h_pre2, g2_sb, e2_sb)

            # ---- final transpose to token-major + store ----
            for c in range(NT // 128):
                o_sb = act_pool.tile([128, D], f32, tag="act")
                for dt in range(DT):
                    pt = pacc.tile([128, 128], f32r, tag="pacc")
                    nc.tensor.transpose(
                        pt[:, :], hT[dt][:, c * 128:(c + 1) * 128], identr[:, :]
                    )
                    nc.scalar.copy(o_sb[:, dt * 128:(dt + 1) * 128], pt[:, :])
                nc.sync.dma_start(OUT[c * 128:(c + 1) * 128, :], o_sb[:, :])

    nc.compile()
    return nc


def _layernorm(nc, act_pool, tmp_pool, uw_pool, sm_pool, pacc, ones_r, eps_sb, h_in, g_sb, b_sb):
    """Post-LN over the feature (partition) dim of feature-major tiles."""
    ms0 = sm_pool.tile([1, NT], f32r, tag="ms", bufs=1)
    ms1 = sm_pool.tile([1, NT], f32r, tag="ms1", bufs=1)
    for nch in range(NT // 512):
        sl = slice(nch * 512, (nch + 1) * 512)
        st0 = pacc.tile([1, 512], f32, tag="pacc", name=_nm("st0"))
        st1 = pacc.tile([1, 512], f32, tag="pacc", name=_nm("st1"))
        for kt in range(DT):
            nc.tensor.matmul(
                st0[0:1, :], ones_r[:, :], h_in[kt][:, sl],
                start=(kt == 0), stop=(kt == DT - 1),
            )
        for kt in range(DT):
            sq = tmp_pool.tile([128, 512], f32r, tag="tmp")
            nc.vector.tensor_mul(sq[:, :], h_in[kt][:, sl], h_in[kt][:, sl])
            nc.tensor.matmul(
                st1[0:1, :], ones_r[:, :], sq[:, :],
                start=(kt == 0), stop=(kt == DT - 1),
            )
        nc.scalar.mul(ms0[:, sl], st0[0:1, :], 1.0 / D)
        nc.scalar.mul(ms1[:, sl], st1[0:1, :], 1.0 / D)
    m2 = sm_pool.tile([1, NT], f32r, tag="lns", bufs=2, name=_nm("m2"))
    nc.vector.tensor_mul(m2[:, :], ms0[:, :], ms0[:, :])
    var = sm_pool.tile([1, NT], f32r, tag="lns", bufs=2, name=_nm("var"))
    nc.vector.tensor_sub(var[:, :], ms1[:, :], m2[:, :])
    sd = sm_pool.tile([1, NT], f32r, tag="lns", bufs=2, name=_nm("sd"))
    nc.scalar.activation(sd[:, :], var[:, :], AF.Sqrt, bias=eps_sb[0:1, 0:1], scale=1.0)
    inv = sm_pool.tile([1, NT], f32r, tag="inv", bufs=1)
    with nc.allow_low_precision(reason="f32r LN inv, ~1e-4 rounding is fine"):
        nc.vector.reciprocal(inv[:, :], sd[:, :])
    w = sm_pool.tile([1, NT], f32r, tag="w", bufs=1)
    nc.vector.tensor_mul(w[:, :], ms0[:, :], inv[:, :])
    U = uw_pool.tile([128, NT], f32r, tag="U")
    nc.gpsimd.partition_broadcast(U[:, :], inv[0:1, :])
    W = uw_pool.tile([128, NT], f32r, tag="W")
    nc.gpsimd.partition_broadcast(W[:, :], w[0:1, :])
    h_out = []
    for dt in range(DT):
        t1 = tmp_pool.tile([128, NT], f32r, tag="tmp")
        nc.vector.tensor_mul(t1[:, :], h_in[dt][:, :], U[:, :])
        nc.vector.tensor_sub(t1[:, :], t1[:, :], W[:, :])
        ho = act_pool.tile([128, NT], f32r, tag="act")
        nc.scalar.activation(
            ho[:, :], t1[:, :], AF.Identity,
            bias=b_sb[:, dt:dt + 1], scale=g_sb[:, dt:dt + 1],
        )
        h_out.append(ho)
    return h_out


def _host_prep(x, batch_length, embed, Wq, Wk, Wv, Wo, bo, ln1_g, ln1_b,
               W1, b1, W2, b2, ln2_g, ln2_b):
    x = np.asarray(x).astype(np.int64)
    batch_length = np.asarray(batch_length).astype(np.int64)
    f = lambda a: np.ascontiguousarray(np.asarray(a), dtype=np.float32)
    embed = f(embed)

    # sinusoidal PE, exact float32 replication of the reference formula
    pos = np.arange(S, dtype=np.float32)[:, None]
    i = np.arange(D, dtype=np.float32)[None, :]
    ang = (pos / np.power(np.float32(10000.0), (np.float32(2.0) * i / np.float32(D)),
                          dtype=np.float32)).astype(np.float32)
    pe = ang.copy()
    pe[:, 0::2] = np.sin(ang[:, 0::2])
    pe[:, 1::2] = np.cos(ang[:, 1::2])

    scale = np.float32(1.0) / np.sqrt(np.float32(Dh))

    # weights, pre-tiled [L, kt, 128, out]
    def tile_k(w):
        w = f(w)
        kt, mt = w.shape[1] // 128, w.shape[2] // 128
        return np.ascontiguousarray(
            w.reshape(L, kt, 128, mt, 128).transpose(0, 1, 3, 2, 4)
        )

    wq = tile_k(np.asarray(Wq) * scale)
    wk = tile_k(Wk)
    wv_full = f(Wv)
    wv = np.ascontiguousarray(wv_full.reshape(L, DT, 128, D))
    wo = tile_k(Wo)
    w1 = tile_k(W1)
    w2 = tile_k(W2)

    tile_b = lambda b, nt: np.ascontiguousarray(
        f(b).reshape(L, nt, 128).transpose(0, 2, 1)
    )
    bo_t = tile_b(bo, DT)
    b1_t = tile_b(b1, FT)
    b2_t = tile_b(b2, DT)
    g1_t = tile_b(ln1_g, DT)
    e1_t = tile_b(ln1_b, DT)
    g2_t = tile_b(ln2_g, DT)
    e2_t = tile_b(ln2_b, DT)

    # mask bias table in blocked pseudo-key order: kb[p, m*4+kt] for jk'=kt*128+p
    pad = (x == 0)
    kb = np.zeros((128, 64), np.float32)
    for m in range(16):
        for kt in range(4):
            jk_blk = kt * 128 + np.arange(128)          # jk' = hk*64+smk
            hk, smk = jk_blk // 64, jk_blk % 64
            jk = smk * 8 + hk                           # interleaved pseudo-pos
            kb[:, m * 4 + kt] = np.where(pad[m, jk], np.float32(-1e30), 0.0)

    in_maps = []
    for c in range(NCORES):
        bsel = slice(2 * c, 2 * c + 2)
        xs = x[bsel]                                    # [2, S]
        lm = (np.arange(S)[None, :] < batch_length[bsel, None]).astype(np.float32)
        pem = (pe.T[None, :, :] * lm[:, None, :])       # [2, D, S]
        pem = pem.transpose(1, 0, 2).reshape(D, NT)     # [D, token=(b,s)]
        pem = np.ascontiguousarray(pem.reshape(DT, 128, NT))
        idx = xs.reshape(NT).astype(np.int16)
        idx = np.ascontiguousarray(np.tile(idx.reshape(NT // 16, 16).T, (8, 1)))
        in_maps.append({
            "IDX": idx, "PEM": pem, "KB": kb, "EMB": embed,
            "WQ": wq, "WK": wk, "WV": wv, "WO": wo, "W1": w1, "W2": w2,
            "BO": bo_t, "B1": b1_t, "B2": b2_t,
            "G1": g1_t, "E1": e1_t, "G2": g2_t, "E2": e2_t,
        })
    return in_maps


def kernel(**inputs):
    if "nc" not in _CACHE:
        _CACHE["nc"] = _build()
    nc = _CACHE["nc"]
    in_maps = _host_prep(**inputs)
    res = None
    for attempt in range(3):
        try:
            res = run_bass_kernel_spmd(nc, in_maps, core_ids=list(range(NCORES)))
            break
        except Exception:
            if attempt == 2:
                raise
    out = np.empty((B, S, D), np.float32)
    for c in range(NCORES):
        out[2 * c:2 * c + 2] = res.results[c]["OUT"].reshape(BPC, S, D)
    return out



# revision 34
# speedup vs baseline: 1.9391x; 1.9391x over previous
"""Trainium2 Bass kernel for the 6-layer post-LN transformer encoder (v2).

Data-parallel over batch: 8 NeuronCores x 2 batches each, weights replicated.
Activations are feature-major ``hT[d, token]`` f32r; score matmuls run bf16.

v2 changes vs the first working kernel:
- padding mask folded into the score matmul as a 65th contraction row
  (k2 row 64 = per-key mask bias, q3 row 64 = ones), so softmax is a single
  big Exp per score tile with no bias table;
- att@V contracts over 128 pseudo-keys at once (V staged as [128, 65] bf16
  per head-pair), halving AV matmuls and removing the odd-head copies;
- softmax denominators via reciprocal_approx_fast + one gpsimd broadcast +
  two strided muls per group (normalized ctx written straight into the
  feature-major ctx tiles Wo consumes);
- LayerNorm 1/sd computed as exp(ln(D) - 0.5*ln(D*sum(h^2) - sum(h)^2 +
  eps*D^2)) so the kernel only ever loads the ln/exp activation table;
- bias+residual fused into single DVE scalar_tensor_tensor ops;
- per-layer weights fetched with a handful of wide DMAs instead of
  per-[128,128]-tile transfers.
"""

import os
import sys

import numpy as np

for _p in ("/opt/trn_rl_repo", "/root/.axon_site/_ro/trn_rl_repo"):
    if os.path.isdir(_p) and _p not in sys.path:
        sys.path.append(_p)

import concourse.bass as bass
import concourse.mybir as mybir
from concourse import bacc
from concourse.tile import TileContext
from concourse.bass_utils import run_bass_kernel_spmd
from concourse.masks import make_identity

B, S, D, H, Dh, F, L, V = 16, 512, 512, 8, 64, 2048, 6, 32000
NCORES = 8
BPC = B // NCORES          # batches per core
NT = BPC * S               # tokens per core
DT = D // 128              # d-dim partition tiles
FT = F // 128              # ffn-dim partition tiles
NG = BPC * 8               # pseudo attention groups per core
f32 = mybir.dt.float32
f32r = mybir.dt.float32r
bf16 = mybir.dt.bfloat16
AF = mybir.ActivationFunctionType
ALU = mybir.AluOpType

_CACHE = {}
_UID = [0]


def _nm(p):
    _UID[0] += 1
    return f"{p}{_UID[0]}"


def _build():
    nc = bacc.Bacc(None, target_bir_lowering=False)

    # Blank every activation-table set except natural_log_exp_and_others so
    # the greedy table-load pass keeps one set resident for the whole kernel
    # (it holds exp, ln, square, relu, identity, copy - everything we use).
    from concourse.hw_specs import get_activation_tables
    _tabs = get_activation_tables(nc.m.arch)
    for _name in list(_tabs):
        if _name != "natural_log_exp_and_others":
            _tabs[_name] = set()

    IDX = nc.dram_tensor("IDX", [128, NT // 16], mybir.dt.int16, kind="ExternalInput")
    PEM = nc.dram_tensor("PEM", [DT, 128, NT], f32, kind="ExternalInput")
    EMB = nc.dram_tensor("EMB", [V, D], f32, kind="ExternalInput")
    KM = nc.dram_tensor("KM", [1, H * NT], bf16, kind="ExternalInput")
    WQ = nc.dram_tensor("WQ", [L, DT, 128, D], f32r, kind="ExternalInput")
    WK = nc.dram_tensor("WK", [L, DT, 128, D], f32r, kind="ExternalInput")
    WV = nc.dram_tensor("WV", [L, DT, 128, D], f32r, kind="ExternalInput")
    WO = nc.dram_tensor("WO", [L, DT, 128, D], f32r, kind="ExternalInput")
    W1 = nc.dram_tensor("W1", [L, FT, 128, D], f32r, kind="ExternalInput")
    W2 = nc.dram_tensor("W2", [L, FT, 128, D], f32r, kind="ExternalInput")
    # packed per-layer small params: bo 0:4 | b1 4:20 | b2 20:24 | g1 24:28
    # | e1 28:32 | g2 32:36 | e2 36:40
    PAR = nc.dram_tensor("PAR", [L, 128, 40], f32, kind="ExternalInput")
    OUT = nc.dram_tensor("OUT", [NT, D], f32, kind="ExternalOutput")

    DBG = os.environ.get("KDBG") == "1"
    if DBG:
        DHT = nc.dram_tensor("DHT", [DT, 128, NT], f32r, kind="ExternalOutput")
        DQ3 = nc.dram_tensor("DQ3", [65, H * NT], bf16, kind="ExternalOutput")
        DK2 = nc.dram_tensor("DK2", [65, H * NT], bf16, kind="ExternalOutput")
        DV2 = nc.dram_tensor("DV2", [128, NG, 4, 65], bf16, kind="ExternalOutput")
        DE = nc.dram_tensor("DE", [128, 2048], bf16, kind="ExternalOutput")
        DCPS = nc.dram_tensor("DCPS", [128, 512], f32, kind="ExternalOutput")
        DCTX = nc.dram_tensor("DCTX", [2, 128, DT, 512], f32r, kind="ExternalOutput")
        DHP = nc.dram_tensor("DHP", [DT, 128, NT], f32r, kind="ExternalOutput")
        DH1 = nc.dram_tensor("DH1", [DT, 128, NT], f32r, kind="ExternalOutput")

    from contextlib import ExitStack

    with TileContext(nc) as tc:
        with ExitStack() as st:
            act_pool = st.enter_context(tc.tile_pool(name="act", bufs=18))
            stk_pool = st.enter_context(tc.tile_pool(name="stk", bufs=1))
            v2_pool = st.enter_context(tc.tile_pool(name="v2p", bufs=1))
            e_pool = st.enter_context(tc.tile_pool(name="ebuf", bufs=4))
            ctx_pool = st.enter_context(tc.tile_pool(name="ctx", bufs=2))
            wqkvo_pool = st.enter_context(tc.tile_pool(name="wqkvo", bufs=10))
            w1_pool = st.enter_context(tc.tile_pool(name="w1p", bufs=4))
            w2_pool = st.enter_context(tc.tile_pool(name="w2p", bufs=4))
            f_pool = st.enter_context(tc.tile_pool(name="fbuf", bufs=3))
            pem_pool = st.enter_context(tc.tile_pool(name="pem", bufs=4))
            rb_pool = st.enter_context(tc.tile_pool(name="rbp", bufs=2))
            tmp_pool = st.enter_context(tc.tile_pool(name="tmp", bufs=3))
            g_pool = st.enter_context(tc.tile_pool(name="gsb", bufs=1))
            sm_pool = st.enter_context(tc.tile_pool(name="small", bufs=2))
            cst_pool = st.enter_context(tc.tile_pool(name="cst", bufs=1))
            pbig = st.enter_context(tc.tile_pool(name="pbig", bufs=3, space="PSUM"))
            psml = st.enter_context(tc.tile_pool(name="psml", bufs=2, space="PSUM"))

            # ---- constants ----
            ident = cst_pool.tile([128, 128], f32, tag="identf")
            make_identity(nc, ident[:, :])
            identr = cst_pool.tile([128, 128], f32r, tag="identr")
            nc.vector.tensor_copy(identr[:, :], ident[:, :])
            ones_f = cst_pool.tile([128, 1], f32, tag="onesf")
            nc.gpsimd.memset(ones_f[:, :], 1.0)
            ones_r = cst_pool.tile([128, 1], f32r, tag="ones")
            nc.vector.tensor_copy(ones_r[:, :], ones_f[:, 0:1])
            onesrow_f = cst_pool.tile([1, 128], f32, tag="onesrowf")
            nc.gpsimd.memset(onesrow_f[:, :], 1.0)
            onesrow_r = cst_pool.tile([1, 128], f32r, tag="onesrow")
            nc.vector.tensor_copy(onesrow_r[:, :], onesrow_f[:, :])
            epsD2 = cst_pool.tile([1, 1], f32, tag="epsD2")
            nc.gpsimd.memset(epsD2[:, :], 1e-5 * D * D)
            lnD = cst_pool.tile([1, 1], f32, tag="lnD")
            nc.gpsimd.memset(lnD[:, :], float(np.log(np.float32(D))))
            idx_sb = cst_pool.tile([128, NT // 16], mybir.dt.int16, tag="idx")
            nc.sync.dma_start(idx_sb[:, :], IDX[:, :])

            # ---- persistent stacks ----
            # q3/k2: [65, H*NT]; rows 0:64 written per layer, row 64 static
            q3 = stk_pool.tile([65, H * NT], bf16, tag="q3")
            k2 = stk_pool.tile([65, H * NT], bf16, tag="k2")
            nc.gpsimd.memset(q3[64:65, :], 1.0)
            nc.sync.dma_start(k2[64:65, :], KM[:, :])
            q3v = q3.rearrange("p (h t) -> p h t", h=H)
            k2v = k2.rearrange("p (h t) -> p h t", h=H)
            # v2: per group [128, 4*65]; ones columns static
            v2 = v2_pool.tile([128, NG, 4, 65], bf16, tag="v2")
            nc.gpsimd.memset(v2[:, :, :, 64:65], 1.0)

            # ---- embedding: gather + transpose + pe add ----
            # activations are per-nch [2][DT] tiles of [128, 512]
            hT = [[act_pool.tile([128, 512], f32r, tag="act", name=_nm("hT"))
                   for _ in range(DT)] for _ in range(2)]
            pem_sb = []
            for dt in range(DT):
                p = pem_pool.tile([128, NT], f32, tag="pem")
                nc.sync.dma_start(p[:, :], PEM[dt, :, :])
                pem_sb.append(p)
            for half in range(2):
                g = g_pool.tile([128, NT // 256, D], f32, tag="gsb")
                nc.gpsimd.dma_gather(
                    g[:, :, :], EMB[:, :],
                    idx_sb[:, half * (NT // 32):(half + 1) * (NT // 32)],
                    NT // 2, NT // 2, D,
                )
                hsl = slice(half * 512, (half + 1) * 512)
                for dt in range(DT):
                    pt = pbig.tile([128, 512], f32, tag="pbig", name=_nm("ept"))
                    for ch in range(NT // 256):
                        nc.tensor.transpose(
                            pt[:, ch * 128:(ch + 1) * 128],
                            g[:, ch, dt * 128:(dt + 1) * 128],
                            ident[:, :],
                        )
                    nc.vector.tensor_add(hT[half][dt][:, :], pt[:, :],
                                         pem_sb[dt][:, hsl])
            if DBG:
                for dt in range(DT):
                    for nch in range(2):
                        nc.sync.dma_start(
                            DHT[dt, :, nch * 512:(nch + 1) * 512],
                            hT[nch][dt][:, :])

            # ---- layers ----
            for l in range(L):
                par = sm_pool.tile([128, 40], f32, tag="par")
                nc.sync.dma_start(par[:, :], PAR[l, :, :])
                bo_c = par[:, 0:4]
                b1_c = par[:, 4:20]
                b2_c = par[:, 20:24]
                g1_c = par[:, 24:28]
                e1_c = par[:, 28:32]
                g2_c = par[:, 32:36]
                e2_c = par[:, 36:40]

                wq_sb, wk_sb, wv_sb, wo_sb = [], [], [], []
                for W_dram, dst in ((WQ, wq_sb), (WK, wk_sb), (WV, wv_sb), (WO, wo_sb)):
                    for kt in range(DT):
                        w = wqkvo_pool.tile([128, D], f32r, tag="wqkvo")
                        nc.sync.dma_start(w[:, :], W_dram[l, kt, :, :])
                        dst.append(w)

                # --- Q/K projections into [65, H*NT] bf16 stacks ---
                for w_sb, stack, eng in ((wq_sb, q3, nc.scalar), (wk_sb, k2, nc.vector)):
                    for mt in range(DT):
                        ps = pbig.tile([128, NT], f32, tag="pbig", name=_nm("pqk"))
                        for kt in range(DT):
                            for nch in range(2):
                                nc.tensor.matmul(
                                    ps[:, nch * 512:(nch + 1) * 512],
                                    w_sb[kt][:, mt * 128:(mt + 1) * 128],
                                    hT[nch][kt][:, :],
                                    start=(kt == 0), stop=(kt == DT - 1),
                                )
                        if eng is nc.scalar:
                            nc.scalar.copy(
                                stack[0:64, (2 * mt) * NT:(2 * mt + 1) * NT],
                                ps[0:64, :])
                            nc.scalar.copy(
                                stack[0:64, (2 * mt + 1) * NT:(2 * mt + 2) * NT],
                                ps[64:128, :])
                        else:
                            nc.vector.tensor_copy(
                                stack[0:64, (2 * mt) * NT:(2 * mt + 1) * NT],
                                ps[0:64, :])
                            nc.vector.tensor_copy(
                                stack[0:64, (2 * mt + 1) * NT:(2 * mt + 2) * NT],
                                ps[64:128, :])

                # --- V projection into v2 [128, g, kt2, 65] bf16 ---
                for tt in range(NT // 128):
                    psv = psml.tile([128, 512], f32, tag="psml", name=_nm("psv"))
                    for kt in range(DT):
                        nc.tensor.matmul(
                            psv[:, :],
                            hT[tt // 4][kt][:, (tt % 4) * 128:(tt % 4 + 1) * 128],
                            wv_sb[kt][:, :],
                            start=(kt == 0), stop=(kt == DT - 1),
                        )
                    psv_v = psv.rearrange("p (h e) -> p h e", h=H)
                    for half in range(2):
                        g = 2 * tt + half
                        sl = slice(half * 64, half * 64 + 64)
                        # even heads -> v2 partitions 0:64, odd -> 64:128
                        eng_copy = nc.scalar.copy if half == 0 else nc.vector.tensor_copy
                        eng_copy(
                            v2[0:64, g, :, 0:64],
                            psv_v[sl, 0::2, :])
                        eng_copy(
                            v2[64:128, g, :, 0:64],
                            psv_v[sl, 1::2, :])

                if DBG and l == 0:
                    nc.sync.dma_start(DQ3[:, :], q3[:, :])
                    nc.sync.dma_start(DK2[:, :], k2[:, :])
                    nc.sync.dma_start(DV2[:, :, :, :], v2[:, :, :, :])

                # --- attention per group ---
                ctx_h = [ctx_pool.tile([128, DT, 512], f32r, tag="ctx", name=_nm("ctx"))
                         for _ in range(2)]
                for g in range(NG):
                    tb = g * 64
                    e_sb = e_pool.tile([128, 2048], bf16, tag="e4", name=_nm("esb"))
                    for p2 in range(2):
                        sp = pbig.tile([128, 1024], f32, tag="pbig", name=_nm("sp"))
                        for j in range(4):
                            hk = p2 * 4 + j
                            nc.tensor.matmul(
                                sp[(hk % 2) * 64:(hk % 2) * 64 + 64,
                                   (j // 2) * 512:(j // 2) * 512 + 512],
                                k2v[:, hk, tb:tb + 64],
                                q3v[:, :, tb:tb + 64],
                                start=True, stop=True,
                            )
                        nc.scalar.activation(
                            e_sb[:, p2 * 1024:(p2 + 1) * 1024], sp[:, :],
                            AF.Exp, scale=1.0)
                    cps = psml.tile([128, 512], f32, tag="psml", name=_nm("cps"))
                    for kt2 in range(4):
                        nc.tensor.matmul(
                            cps[0:65, :],
                            v2[:, g, kt2, :],
                            e_sb[:, kt2 * 512:(kt2 + 1) * 512],
                            start=(kt2 == 0), stop=(kt2 == 3),
                        )
                    if DBG and l == 0 and g == 0:
                        nc.sync.dma_start(DE[:, :], e_sb[:, :])
                        dcp = tmp_pool.tile([128, 512], f32, tag="osb", bufs=2)
                        nc.scalar.copy(dcp[:, :], cps[:, :])
                        nc.sync.dma_start(DCPS[:, :], dcp[:, :])
                    den_f = sm_pool.tile([1, 512], f32, tag="denf", bufs=2)
                    nc.vector.tensor_copy(den_f[:, :], cps[64:65, :])
                    den = sm_pool.tile([1, 512], f32, tag="den", bufs=2)
                    nc.vector.reciprocal_approx_fast(den[:, :], den_f[:, :])
                    rb = rb_pool.tile([64, 512], f32, tag="rb")
                    nc.gpsimd.partition_broadcast(rb[:, :], den[0:1, :])
                    rb_v = rb.rearrange("p (h q) -> p h q", h=8)
                    cps_v = cps.rearrange("p (h q) -> p h q", h=8)
                    ctx_t = ctx_h[g // 8]
                    tl = tb % 512
                    nc.vector.tensor_mul(
                        ctx_t[0:64, :, tl:tl + 64],
                        cps_v[0:64, 0::2, :],
                        rb_v[:, 0::2, :])
                    nc.vector.tensor_mul(
                        ctx_t[64:128, :, tl:tl + 64],
                        cps_v[0:64, 1::2, :],
                        rb_v[:, 1::2, :])

                # --- Wo + bias + residual -> h_pre ---
                h_pre = [[act_pool.tile([128, 512], f32r, tag="act",
                                        name=_nm("hpre")) for _ in range(DT)]
                         for _ in range(2)]
                for mt in range(DT):
                    pso = pbig.tile([128, NT], f32, tag="pbig", name=_nm("pso"))
                    for kt in range(DT):
                        for nch in range(2):
                            nc.tensor.matmul(
                                pso[:, nch * 512:(nch + 1) * 512],
                                wo_sb[kt][:, mt * 128:(mt + 1) * 128],
                                ctx_h[nch][:, kt, :],
                                start=(kt == 0), stop=(kt == DT - 1),
                            )
                    for nch in range(2):
                        nc.vector.scalar_tensor_tensor(
                            h_pre[nch][mt][:, :], pso[:, nch * 512:(nch + 1) * 512],
                            bo_c[:, mt:mt + 1], hT[nch][mt][:, :],
                            op0=ALU.add, op1=ALU.add)

                if DBG and l == 0:
                    for nch in range(2):
                        nc.sync.dma_start(
                            DCTX[nch, :, :, :], ctx_h[nch][:, :, :])
                    for dt in range(DT):
                        for nch in range(2):
                            nc.sync.dma_start(
                                DHP[dt, :, nch * 512:(nch + 1) * 512],
                                h_pre[nch][dt][:, :])

                h1 = _layernorm(nc, act_pool, tmp_pool, sm_pool, psml,
                                ones_r, onesrow_r, epsD2, lnD, h_pre, g1_c, e1_c)
                if DBG and l == 0:
                    for dt in range(DT):
                        for nch in range(2):
                            nc.sync.dma_start(
                                DH1[dt, :, nch * 512:(nch + 1) * 512],
                                h1[nch][dt][:, :])

                # --- FFN ---
                h_pre2 = [[act_pool.tile([128, 512], f32r, tag="act",
                                         name=_nm("hpre2")) for _ in range(DT)]
                          for _ in range(2)]
                for nch in range(2):
                    nsl = slice(nch * 512, (nch + 1) * 512)
                    o2b = [pbig.tile([128, 1024], f32, tag="pbig", name=_nm("o2b"))
                           for _ in range(2)]
                    o2ps = [o2b[mtd // 2][:, (mtd % 2) * 512:(mtd % 2 + 1) * 512]
                            for mtd in range(DT)]
                    for mtf in range(FT):
                        w1t = w1_pool.tile([128, D], f32r, tag="w1")
                        nc.sync.dma_start(w1t[:, :], W1[l, mtf, :, :])
                        w2t = w2_pool.tile([128, D], f32r, tag="w2")
                        nc.sync.dma_start(w2t[:, :], W2[l, mtf, :, :])
                        psf = psml.tile([128, 512], f32, tag="psml", name=_nm("psf"))
                        for kt in range(DT):
                            nc.tensor.matmul(
                                psf[:, :],
                                w1t[:, kt * 128:(kt + 1) * 128],
                                h1[nch][kt][:, :],
                                start=(kt == 0), stop=(kt == DT - 1),
                            )
                        f_t = f_pool.tile([128, 512], f32r, tag="f")
                        nc.scalar.activation(
                            f_t[:, :], psf[:, :], AF.Relu,
                            bias=b1_c[:, mtf:mtf + 1], scale=1.0)
                        for mtd in range(DT):
                            nc.tensor.matmul(
                                o2ps[mtd][:, :],
                                w2t[:, mtd * 128:(mtd + 1) * 128],
                                f_t[:, :],
                                start=(mtf == 0), stop=(mtf == FT - 1),
                            )
                    for mtd in range(DT):
                        nc.vector.scalar_tensor_tensor(
                            h_pre2[nch][mtd][:, :], o2ps[mtd][:, :],
                            b2_c[:, mtd:mtd + 1], h1[nch][mtd][:, :],
                            op0=ALU.add, op1=ALU.add)

                hT = _layernorm(nc, act_pool, tmp_pool, sm_pool, psml,
                                ones_r, onesrow_r, epsD2, lnD, h_pre2, g2_c, e2_c)

            # ---- final transpose to token-major + store ----
            for c in range(NT // 128):
                pt = psml.tile([128, 512], f32r, tag="psml", name=_nm("fpt"))
                for dt in range(DT):
                    nc.tensor.transpose(
                        pt[:, dt * 128:(dt + 1) * 128],
                        hT[c // 4][dt][:, (c % 4) * 128:(c % 4 + 1) * 128],
                        identr[:, :])
                o_sb = tmp_pool.tile([128, D], f32, tag="osb", bufs=2)
                nc.scalar.copy(o_sb[:, :], pt[:, :])
                nc.sync.dma_start(OUT[c * 128:(c + 1) * 128, :], o_sb[:, :])

    nc.compile()
    return nc


def _layernorm(nc, act_pool, tmp_pool, sm_pool, psml,
               ones_r, onesrow_r, epsD2, lnD, h_in, g_c, b_c):
    """Post-LN over the feature (partition) dim of per-nch feature-major
    tiles.  inv = 1/sd via exp(ln(D) - 0.5*ln(D*st1 - st0^2 + eps*D^2));
    U/W rows broadcast to 128 partitions with K=1 PE matmuls."""
    h_out = [[act_pool.tile([128, 512], f32r, tag="act", name=_nm("ho"))
              for _ in range(DT)] for _ in range(2)]
    for nch in range(2):
        st0 = psml.tile([1, 512], f32, tag="psml", name=_nm("st0"))
        st1 = psml.tile([1, 512], f32, tag="psml", name=_nm("st1"))
        for kt in range(DT):
            nc.tensor.matmul(
                st0[0:1, :], ones_r[:, :], h_in[nch][kt][:, :],
                start=(kt == 0), stop=(kt == DT - 1),
            )
        for kt in range(DT):
            sq = tmp_pool.tile([128, 512], f32r, tag="tmp")
            nc.vector.tensor_mul(sq[:, :], h_in[nch][kt][:, :], h_in[nch][kt][:, :])
            nc.tensor.matmul(
                st1[0:1, :], ones_r[:, :], sq[:, :],
                start=(kt == 0), stop=(kt == DT - 1),
            )
        sq0 = sm_pool.tile([1, 512], f32, tag="lnr", bufs=2, name=_nm("sq0"))
        nc.scalar.activation(sq0[:, :], st0[0:1, :], AF.Square, scale=1.0)
        tv = sm_pool.tile([1, 512], f32, tag="lnr", bufs=2, name=_nm("tv"))
        nc.vector.scalar_tensor_tensor(
            tv[:, :], st1[0:1, :], float(D), sq0[:, :],
            op0=ALU.mult, op1=ALU.subtract)
        lnt = sm_pool.tile([1, 512], f32, tag="lnr", bufs=2, name=_nm("lnt"))
        nc.scalar.activation(lnt[:, :], tv[:, :], AF.Ln,
                             bias=epsD2[0:1, 0:1], scale=1.0)
        inv = sm_pool.tile([1, 512], f32r, tag="lnrr", bufs=2, name=_nm("inv"))
        nc.scalar.activation(inv[:, :], lnt[:, :], AF.Exp,
                             bias=lnD[0:1, 0:1], scale=-0.5)
        w = sm_pool.tile([1, 512], f32r, tag="lnrr", bufs=2, name=_nm("w"))
        nc.vector.scalar_tensor_tensor(
            w[:, :], st0[0:1, :], 1.0 / D, inv[:, :],
            op0=ALU.mult, op1=ALU.mult)
        U_ps = psml.tile([128, 512], f32, tag="psml", name=_nm("Ups"))
        nc.tensor.matmul(U_ps[:, :], onesrow_r[:, :], inv[:, :],
                         start=True, stop=True)
        W_ps = psml.tile([128, 512], f32, tag="psml", name=_nm("Wps"))
        nc.tensor.matmul(W_ps[:, :], onesrow_r[:, :], w[:, :],
                         start=True, stop=True)
        for dt in range(DT):
            t1 = tmp_pool.tile([128, 512], f32r, tag="tmp")
            nc.vector.tensor_mul(t1[:, :], h_in[nch][dt][:, :], U_ps[:, :])
            t2 = tmp_pool.tile([128, 512], f32r, tag="tmp")
            nc.vector.tensor_sub(t2[:, :], t1[:, :], W_ps[:, :])
            nc.vector.tensor_scalar(
                h_out[nch][dt][:, :], t2[:, :], g_c[:, dt:dt + 1],
                b_c[:, dt:dt + 1], op0=ALU.mult, op1=ALU.add)
    return h_out


def _host_prep(x, batch_length, embed, Wq, Wk, Wv, Wo, bo, ln1_g, ln1_b,
               W1, b1, W2, b2, ln2_g, ln2_b):
    x = np.asarray(x).astype(np.int64)
    batch_length = np.asarray(batch_length).astype(np.int64)
    f = lambda a: np.ascontiguousarray(np.asarray(a), dtype=np.float32)
    embed = f(embed)

    # sinusoidal PE, exact float32 replication of the reference formula
    pos = np.arange(S, dtype=np.float32)[:, None]
    i = np.arange(D, dtype=np.float32)[None, :]
    ang = (pos / np.power(np.float32(10000.0), (np.float32(2.0) * i / np.float32(D)),
                          dtype=np.float32)).astype(np.float32)
    pe = ang.copy()
    pe[:, 0::2] = np.sin(ang[:, 0::2])
    pe[:, 1::2] = np.cos(ang[:, 1::2])

    scale = np.float32(1.0) / np.sqrt(np.float32(Dh))

    # weights pre-tiled [L, kt, 128, out]
    def tile_k(w):
        w = f(w)
        kt = w.shape[1] // 128
        return np.ascontiguousarray(w.reshape(L, kt, 128, w.shape[2]))

    wq = tile_k(np.asarray(Wq) * scale)
    wk = tile_k(Wk)
    wv = tile_k(Wv)
    wo = tile_k(Wo)
    w1f = f(W1)  # [L, D, F]
    w1 = np.ascontiguousarray(
        w1f.reshape(L, DT, 128, FT, 128).transpose(0, 3, 2, 1, 4).reshape(L, FT, 128, D))
    w2 = tile_k(W2)

    tile_b = lambda b_, nt: f(b_).reshape(L, nt, 128).transpose(0, 2, 1)
    par = np.zeros((L, 128, 40), np.float32)
    par[:, :, 0:4] = tile_b(bo, DT)
    par[:, :, 4:20] = tile_b(b1, FT)
    par[:, :, 20:24] = tile_b(b2, DT)
    par[:, :, 24:28] = tile_b(ln1_g, DT)
    par[:, :, 28:32] = tile_b(ln1_b, DT)
    par[:, :, 32:36] = tile_b(ln2_g, DT)
    par[:, :, 36:40] = tile_b(ln2_b, DT)
    par = np.ascontiguousarray(par)

    # mask row: km[hk*NT + t] = -1e30 if pad[m(g), smk*8+hk], t = g*64+smk
    pad = (x == 0)
    km = np.zeros((1, H * NT), np.float32)
    t_idx = np.arange(NT)
    g_idx = t_idx // 64
    smk = t_idx % 64
    b_local = t_idx // 512
    gb = (t_idx % 512) // 64
    m_row = (8 * b_local + gb) % 16
    for hk in range(H):
        jk = smk * 8 + hk
        km[0, hk * NT + t_idx] = np.where(pad[m_row, jk], np.float32(-1e30), 0.0)
    import ml_dtypes
    km = km.astype(ml_dtypes.bfloat16)

    in_maps = []
    for c in range(NCORES):
        bsel = slice(2 * c, 2 * c + 2)
        xs = x[bsel]                                    # [2, S]
        lm = (np.arange(S)[None, :] < batch_length[bsel, None]).astype(np.float32)
        pem = (pe.T[None, :, :] * lm[:, None, :])       # [2, D, S]
        pem = pem.transpose(1, 0, 2).reshape(D, NT)     # [D, token=(b,s)]
        pem = np.ascontiguousarray(pem.reshape(DT, 128, NT))
        idx = xs.reshape(NT).astype(np.int16)
        idx = np.ascontiguousarray(np.tile(idx.reshape(NT // 16, 16).T, (8, 1)))
        in_maps.append({
            "IDX": idx, "PEM": pem, "KM": km, "EMB": embed,
            "WQ": wq, "WK": wk, "WV": wv, "WO": wo, "W1": w1, "W2": w2,
            "PAR": par,
        })
    return in_maps


def kernel(**inputs):
    if "nc" not in _CACHE:
        _CACHE["nc"] = _build()
    nc = _CACHE["nc"]
    in_maps = _host_prep(**inputs)
    res = None
    for attempt in range(3):
        try:
            res = run_bass_kernel_spmd(nc, in_maps, core_ids=list(range(NCORES)))
            break
        except Exception:
            if attempt == 2:
                raise
    _CACHE["res"] = res
    out = np.empty((B, S, D), np.float32)
    for c in range(NCORES):
        out[2 * c:2 * c + 2] = res.results[c]["OUT"].reshape(BPC, S, D)
    return out
